# revision 7
# baseline (speedup 1.0000x reference)
"""VoGE mesh-interpolate (Gaussian splat + volume blend) Trainium2 kernel.

Strategy
--------
Host (numpy, tiny: O(N) work for N=1024 Gaussians):
  * preprocess mesh (axis swap), per-vertex variance from mean incident edge
    length, camera R/T, project to NDC, per-camera front-to-back depth sort.
  * pack per-Gaussian quadratic coefficients so the per-pixel Mahalanobis
    distance q = d^2/(2*s2d) becomes a small matmul:
       q[n,p] = Bm[0,n]*gx_p + Bm[1,n]*gy_p + Bm[2,n]*(gx_p^2+gy_p^2) + Bm[3,n]
    For speed this runs as a K=16 bf16 matmul with hi/lo-split operands
    (q = (Bh+Bl)^T(ph+pl), all four cross terms), accurate to ~1e-5 rel.
  * behind-camera culling folded into Bm[3] as a +1e9 bias (w -> 0).

Device (8 cores; core i owns image rows 16i..16i+15 = 2048 pixels, both
cameras; each core holds the full Gaussian set):
  for each (camera b, pixel-block of F=512):
    for each chunk k of 128 depth-sorted Gaussians:
      PE : cums  = Bm16_k^T @ pix16        (q, K=16 bf16 matmul, psum group)
      ACT: w     = exp(-cums)              (psum -> sbuf)
      PE : cums += U_strict^T @ w          (exclusive cumsum along partitions)
      PE : cums += ones128^T  @ carry      (adds per-pixel carry of previous
                                            chunks' total density, k>0)
      ACT: wf    = exp(-cums)              (= w * transmittance)
      PE : acc  += feat_k^T @ wf           (feature blend, [C=128, F])
      PE : nrm  += ones^T   @ wf           (normalizer row)
      DVE: carry += w                      (running density total)
    copy acc psum -> sbuf, DMA out; DMA nrm row out.
Host: out = acc / (nrm + 1e-8), reassemble [B,H,W,C].

Precision configs (env KCONFIG): "fast" = cumsum/blend matmuls in float32r
(~10-bit mantissa), "mixed" = cumsum matmuls fp32, blend f32r, "fp32" = all
fp32 (4x slower PE).
"""

import os

import numpy as np

B, N, NFACES, C, HW = 2, 1024, 2048, 128, 128
FOCAL = 2.0
NCORES = 8
ROWS_PER_CORE = HW // NCORES  # 16
PIX_PER_CORE = ROWS_PER_CORE * HW  # 2048
NCHUNK = N // 128  # 8


# ----------------------------------------------------------------------------
# host-side preprocessing
# ----------------------------------------------------------------------------
def _split_bf16(x):
    import ml_dtypes

    hi = x.astype(ml_dtypes.bfloat16)
    lo = (x - hi.astype(np.float32)).astype(ml_dtypes.bfloat16)
    return hi, lo


def _host_prep(campos, theta, vertices, faces, memory):
    f32 = np.float32
    campos = np.asarray(campos, f32)
    theta = np.asarray(theta, f32)
    vertices = np.asarray(vertices, f32)
    faces = np.asarray(faces)
    memory = np.asarray(memory, f32)

    verts = vertices[:, [0, 2, 1]]

    i0, i1, i2 = faces[:, 0], faces[:, 1], faces[:, 2]
    e01 = np.linalg.norm(verts[i0] - verts[i1], axis=-1).astype(f32)
    e12 = np.linalg.norm(verts[i1] - verts[i2], axis=-1).astype(f32)
    e20 = np.linalg.norm(verts[i2] - verts[i0], axis=-1).astype(f32)
    idx = np.concatenate([i0, i1, i1, i2, i2, i0])
    val = np.concatenate([e01, e01, e12, e12, e20, e20]).astype(f32)
    ssum = np.zeros(N, f32)
    np.add.at(ssum, idx, val)
    cnt = np.zeros(N, f32)
    np.add.at(cnt, idx, np.ones_like(val))
    mean_edge = ssum / np.maximum(cnt, 1.0)
    radius = (0.5 * mean_edge).astype(f32)
    sigma = radius * radius + np.float32(1e-6)  # [N] variance

    def normalize(v, eps=1e-8):
        return v / (np.linalg.norm(v, axis=-1, keepdims=True).astype(f32) + f32(eps))

    up = np.array([[0.0, 1.0, 0.0]], f32)
    z_ax = normalize(-campos)
    x_ax = normalize(np.cross(np.broadcast_to(up, z_ax.shape), z_ax))
    y_ax = np.cross(z_ax, x_ax)
    R = np.stack([x_ax, y_ax, z_ax], axis=-1).astype(f32)  # [B,3,3]
    cth, sth = np.cos(theta), np.sin(theta)
    zero = np.zeros_like(cth)
    one = np.ones_like(cth)
    Rz = np.stack(
        [
            np.stack([cth, -sth, zero], -1),
            np.stack([sth, cth, zero], -1),
            np.stack([zero, zero, one], -1),
        ],
        axis=-2,
    ).astype(f32)
    R = np.einsum("bij,bjk->bik", R, Rz).astype(f32)
    T = -np.einsum("bji,bj->bi", R, campos).astype(f32)

    v_cam = np.einsum("ni,bij->bnj", verts, R).astype(f32) + T[:, None, :]
    z = v_cam[..., 2]
    zc = np.maximum(z, f32(1e-4))
    px = (FOCAL * v_cam[..., 0] / zc).astype(f32)
    py = (FOCAL * v_cam[..., 1] / zc).astype(f32)
    s2d = (sigma[None, :] * (FOCAL / zc) ** 2).astype(f32)

    order = np.argsort(z, axis=1, kind="stable")
    take = lambda a: np.take_along_axis(a, order, axis=1)
    psx, psy, s2d_s, zs = take(px), take(py), take(s2d), take(z)
    feat_s = memory[order]  # [B,N,C]

    inv2s = (1.0 / (2.0 * s2d_s)).astype(f32)
    r0 = -2.0 * psx * inv2s
    r1 = -2.0 * psy * inv2s
    r2 = inv2s
    r3 = (psx**2 + psy**2) * inv2s + np.where(zs > 1e-4, f32(0), f32(1e9))
    BmT = np.ascontiguousarray(
        np.stack([r0, r1, r2, r3], axis=1).astype(f32).transpose(1, 0, 2).reshape(4, B * N)
    )  # [4, B*N], column b*N + n

    # featT_dev[m, (b*8+k)*128 + c] = feat_s[b, k*128+m, c]
    featT = np.ascontiguousarray(
        feat_s.reshape(B, NCHUNK, 128, C).transpose(2, 0, 1, 3).reshape(128, B * NCHUNK * C)
    )

    # per-core pixel features
    lin = np.linspace(-1.0, 1.0, HW, dtype=f32)
    gy, gx = np.meshgrid(lin, lin, indexing="ij")  # gy[h,w]=lin[h], gx[h,w]=lin[w]
    pixT_cores = []
    for i in range(NCORES):
        gxs = gx[i * ROWS_PER_CORE : (i + 1) * ROWS_PER_CORE].reshape(-1)
        gys = gy[i * ROWS_PER_CORE : (i + 1) * ROWS_PER_CORE].reshape(-1)
        pixT_cores.append(
            np.ascontiguousarray(
                np.stack([gxs, gys, gxs**2 + gys**2, np.ones_like(gxs)], axis=0)
            ).astype(f32)
        )  # [4, 2048]

    # bf16 hi/lo split stacks for the q matmul:
    # q = Bh.ph + Bh.pl + Bl.ph + Bl.pl  ->  K=16 rows [Bh,Bh,Bl,Bl] x [ph,pl,ph,pl]
    Bh, Bl = _split_bf16(BmT)
    Bm16 = np.ascontiguousarray(np.concatenate([Bh, Bh, Bl, Bl], axis=0))  # [16, B*N]
    pix16_cores = []
    for p4 in pixT_cores:
        ph, pl = _split_bf16(p4)
        pix16_cores.append(
            np.ascontiguousarray(np.concatenate([ph, pl, ph, pl], axis=0))
        )  # [16, 2048]

    return BmT, featT, pixT_cores, Bm16, pix16_cores


# ----------------------------------------------------------------------------
# device kernel
# ----------------------------------------------------------------------------
def _build_kernel(cfg="fast", F=512, repeat=1):
    import contextlib

    import concourse.bacc as bacc
    import concourse.mybir as mybir
    import concourse.tile as tile

    f32 = mybir.dt.float32
    f32r = mybir.dt.float32r
    bf16 = mybir.dt.bfloat16
    EXP = mybir.ActivationFunctionType.Exp

    # dtype of the cumsum matmuls (U, A1, w, carry) and blend matmuls
    dt_cum = {"fast": f32r, "mixed": f32, "fp32": f32}[cfg]
    dt_blend = {"fast": f32r, "mixed": f32r, "fp32": f32}[cfg]
    use_bf16_q = cfg in ("fast", "mixed")

    nc = bacc.Bacc("TRN2", target_bir_lowering=False, debug=False)

    if use_bf16_q:
        Bm_d = nc.dram_tensor("Bm16", (16, B * N), bf16, kind="ExternalInput")
        pix_d = nc.dram_tensor("pix16", (16, PIX_PER_CORE), bf16, kind="ExternalInput")
        KQ = 16
        dt_q = bf16
    else:
        Bm_d = nc.dram_tensor("BmT", (4, B * N), f32, kind="ExternalInput")
        pix_d = nc.dram_tensor("pixT", (4, PIX_PER_CORE), f32, kind="ExternalInput")
        KQ = 4
        dt_q = f32
    featT_d = nc.dram_tensor("featT", (128, B * NCHUNK * C), dt_blend, kind="ExternalInput")
    U_d = nc.dram_tensor("Umat", (128, 128), dt_cum, kind="ExternalInput")
    A1_d = nc.dram_tensor("Aones", (128, 128), dt_cum, kind="ExternalInput")
    O1_d = nc.dram_tensor("Cones", (128, 1), dt_blend, kind="ExternalInput")
    acc_d = nc.dram_tensor("out_acc", (B * C, PIX_PER_CORE), f32, kind="ExternalOutput")
    nrm_d = nc.dram_tensor("out_nrm", (B, PIX_PER_CORE), f32, kind="ExternalOutput")

    nblk = PIX_PER_CORE // F

    with tile.TileContext(nc) as tc:
        with (
            tc.tile_pool(name="consts", bufs=1) as consts,
            tc.tile_pool(name="wpool", bufs=3) as wpool,
            tc.tile_pool(name="wfpool", bufs=3) as wfpool,
            tc.tile_pool(name="carryp", bufs=2) as carryp,
            tc.tile_pool(name="outp", bufs=2) as outp,
            tc.tile_pool(name="cumsp", bufs=2, space="PSUM") as cumsp,
            tc.tile_pool(name="accp", bufs=2, space="PSUM") as accp,
            tc.tile_pool(name="nrmp", bufs=2, space="PSUM") as nrmp,
        ):
            pix_sb = consts.tile([KQ, PIX_PER_CORE], dt_q)
            Bm_sb = consts.tile([KQ, B * N], dt_q)
            feat_sb = consts.tile([128, B * NCHUNK * C], dt_blend)
            U_sb = consts.tile([128, 128], dt_cum)
            A1_sb = consts.tile([128, 128], dt_cum)
            O1_sb = consts.tile([128, 1], dt_blend)
            nc.sync.dma_start(out=pix_sb[:], in_=pix_d[:])
            nc.sync.dma_start(out=Bm_sb[:], in_=Bm_d[:])
            nc.sync.dma_start(out=feat_sb[:], in_=featT_d[:])
            nc.sync.dma_start(out=U_sb[:], in_=U_d[:])
            nc.sync.dma_start(out=A1_sb[:], in_=A1_d[:])
            nc.sync.dma_start(out=O1_sb[:], in_=O1_d[:])

            rep_ctx = (
                tc.For_i(0, repeat, 1) if repeat > 1 else contextlib.nullcontext()
            )
            with rep_ctx:
              for b in range(B):
                for blk in range(nblk):
                    pix_blk = pix_sb[:, blk * F : (blk + 1) * F]
                    acc = accp.tile([128, F], f32)
                    nrm = nrmp.tile([1, F], f32)
                    carry = carryp.tile([128, F], dt_cum)
                    for k in range(NCHUNK):
                        cums = cumsp.tile([128, F], f32)
                        nc.tensor.matmul(
                            cums[:],
                            Bm_sb[:, b * N + k * 128 : b * N + (k + 1) * 128],
                            pix_blk,
                            start=True,
                            stop=False,
                        )
                        w = wpool.tile([128, F], dt_cum)
                        nc.scalar.activation(w[:], cums[:], EXP, scale=-1.0)
                        nc.tensor.matmul(
                            cums[:], U_sb[:], w[:], start=False, stop=(k == 0)
                        )
                        if k > 0:
                            nc.tensor.matmul(
                                cums[:], A1_sb[:], carry[:], start=False, stop=True
                            )
                        wf = wfpool.tile([128, F], dt_blend)
                        nc.scalar.activation(wf[:], cums[:], EXP, scale=-1.0)
                        nc.tensor.matmul(
                            acc[:],
                            feat_sb[:, (b * NCHUNK + k) * C : (b * NCHUNK + k + 1) * C],
                            wf[:],
                            start=(k == 0),
                            stop=(k == NCHUNK - 1),
                        )
                        nc.tensor.matmul(
                            nrm[:], O1_sb[:], wf[:], start=(k == 0), stop=(k == NCHUNK - 1)
                        )
                        if k == 0:
                            nc.vector.tensor_copy(carry[:], w[:])
                        elif k < NCHUNK - 1:
                            nc.vector.tensor_add(carry[:], carry[:], w[:])
                    out_sb = outp.tile([128, F], f32)
                    nc.vector.tensor_copy(out_sb[:], acc[:])
                    nc.sync.dma_start(
                        out=acc_d[b * C : (b + 1) * C, blk * F : (blk + 1) * F],
                        in_=out_sb[:],
                    )
                    nrm_sb = outp.tile([1, F], f32, tag="nrm_sb")
                    nc.vector.tensor_copy(nrm_sb[:], nrm[:])
                    nc.sync.dma_start(
                        out=nrm_d[b : b + 1, blk * F : (blk + 1) * F], in_=nrm_sb[:]
                    )
    nc.compile()
    return nc


# ----------------------------------------------------------------------------
# entry point
# ----------------------------------------------------------------------------
def kernel(campos, theta, vertices, faces, memory):
    from concourse.bass_utils import run_bass_kernel_spmd

    f32 = np.float32
    cfg = os.environ.get("KCONFIG", "fast")
    BmT, featT, pixT_cores, Bm16, pix16_cores = _host_prep(
        campos, theta, vertices, faces, memory
    )
    U = np.ascontiguousarray(np.triu(np.ones((128, 128), f32), k=1))
    A1 = np.ones((128, 128), f32)
    O1 = np.ones((128, 1), f32)

    nc = _build_kernel(cfg)

    in_maps = []
    for i in range(NCORES):
        m = {
            "featT": featT,
            "Umat": U,
            "Aones": A1,
            "Cones": O1,
        }
        if cfg in ("fast", "mixed"):
            m["Bm16"] = Bm16
            m["pix16"] = pix16_cores[i]
        else:
            m["BmT"] = BmT
            m["pixT"] = pixT_cores[i]
        in_maps.append(m)

    trace = os.environ.get("KERNEL_TRACE", "0") == "1"
    res = run_bass_kernel_spmd(nc, in_maps, core_ids=list(range(NCORES)), trace=trace)
    if trace and res.exec_time_ns is not None:
        print(f"HW exec time: {res.exec_time_ns} ns")
        if res.instructions_and_trace is not None:
            print("trace:", res.instructions_and_trace[1])

    acc_all = np.stack([r["out_acc"] for r in res.results])  # [8, B*C, 2048]
    nrm_all = np.stack([r["out_nrm"] for r in res.results])  # [8, B, 2048]

    acc = acc_all.reshape(NCORES, B, C, ROWS_PER_CORE, HW)
    num = acc.transpose(1, 0, 3, 4, 2).reshape(B, HW, HW, C)
    nrm = nrm_all.reshape(NCORES, B, ROWS_PER_CORE, HW).transpose(1, 0, 2, 3).reshape(
        B, HW, HW
    )
    out = num / (nrm[..., None] + np.float32(1e-8))
    return out.astype(f32)
